# revision 1
# baseline (speedup 1.0000x reference)
"""Bass/Trainium2 kernel for nn_BiGRIL (gnn_message_passing).

Key algebraic structure (valid because the reference's hidden state h is
identically zero and C == 1):
  x1   = where(mask, x, b_fs)
  z    = W0*x1 + W1*m + b_in            (rank-2 in channels!)
  zg   = A^T z  ->  W0*xg + W1*mg + b_in*cg   with xg = A^T x1, mg = A^T m,
                                              cg = A^T 1
  o    = PReLU(M1 z + M2 zg + b_fold)   (K=6 matmul over 6 data streams)
  xs2  = w_ro . o + b_ro
  y    = relu(W_o1 xs2 + b_o1)          (rank-1 K=64 matmul, relu fused)
  out  = W_o2 . y + b_o2                (K=64 contraction)

Sharding: pure data-parallel over batch (B=8 -> 8 cores), weights + adj
replicated.  No collectives.
"""

import numpy as np
import sys

sys.path.insert(0, "/opt/trn_rl_repo")

B, C, N, T = 8, 1, 1024, 64
H = 64
NT = N * T          # 65536 per-core output elements
CHUNK = 512         # psum-bank-sized column chunk
NCHUNK = NT // CHUNK
BLK = 4096          # movA tile columns (64 nodes x 64 steps)
NBLK = NT // BLK    # 16 blocks per core

_CACHE = {}


def _fold_weights(W_fs, b_fs, W_in, b_in, W_gc, b_gc, W_lo, b_lo, prelu_a,
                  W_ro, b_ro, W_o1, b_o1, W_o2, b_o2, adj):
    """Host-side weight folding in float64 for accuracy."""
    f8 = np.float64
    W_in, b_in = W_in.astype(f8), b_in.astype(f8)
    W_gc, b_gc = W_gc.astype(f8), b_gc.astype(f8)
    W_lo, b_lo = W_lo.astype(f8), b_lo.astype(f8)
    W_ro, b_ro = W_ro.astype(f8), b_ro.astype(f8)
    W_o1, b_o1 = W_o1.astype(f8), b_o1.astype(f8)
    W_o2, b_o2 = W_o2.astype(f8), b_o2.astype(f8)

    W0 = W_in[:, 0]           # x1 channel  [64]
    W1 = W_in[:, 1]           # mask channel [64]
    Wlo1 = W_lo[:, :H]
    M1 = Wlo1 @ W_gc[:, :H]
    M2 = Wlo1 @ W_gc[:, H:]
    b_fold = Wlo1 @ b_gc + b_lo

    PA = np.stack([
        M1 @ W0,
        M1 @ W1,
        M2 @ W0,
        M2 @ W1,
        M2 @ b_in,
        M1 @ b_in + b_fold,
    ])                                     # [6, 64]  lhsT for pass A

    w_ro1 = W_ro[0, :H]                    # [64]
    PB = np.outer(w_ro1, W_o1[:, 0])       # [64(h), 64(f)] lhsT for pass B
    bias_f = W_o1[:, 0] * b_ro[0] + b_o1   # [64]

    den = float(np.sum(W_o2[0] ** 2))
    if abs(den) < 1e-12:
        k = np.zeros(H)
        extra_const = float(b_o2[0])       # would need separate handling
    else:
        k = float(b_o2[0]) * W_o2[0] / den
        extra_const = 0.0
    assert extra_const == 0.0

    cg = adj.astype(f8).sum(axis=0)        # [N] column sums of adj
    cgrep = np.repeat(cg, T)               # [(m,t)] layout m*T + t

    a = float(prelu_a)
    assert 0.0 < a < 1.0
    # All stationaries are K=128 (zero-padded): the PE activity monitor
    # only un-throttles the clock for full-K matmuls.
    # moving layout (ma2): rows 0:6 = streams, 6:64 = zeros,
    #                      64:128 = t1 = (1-a)*relu(v)
    pa128 = np.zeros((128, H))
    pa128[0:6, :] = PA                      # phase A: v = PA^T streams
    pb128 = np.zeros((128, 128))
    pb128[0:6, 0:H] = a * (PA @ PB)         # linear PReLU branch
    pb128[64:128, 0:H] = PB                 # + PB^T t1
    # cols 64:128 stay zero -> psum_v rows 64:128 written as exact zeros
    pc128 = np.zeros((128, 32))
    pc128[0:H, 0] = W_o2[0]
    bk128 = np.zeros((128, 1))
    bk128[:H, 0] = bias_f + k
    kk128 = np.zeros((128, 1))
    kk128[:H, 0] = k

    fp = np.float32
    h16 = np.float16
    return dict(
        pa=pa128.astype(h16),
        pb=pb128.astype(h16),
        pc=pc128.astype(h16),
        bk=bk128.astype(fp),
        kk=kk128.astype(fp),
        bfs=np.full((128, 1), b_fs[0], fp),
        zr=np.zeros((122, BLK), h16),
        sc=np.full((128, 1), 1.0 - a, fp),
        cgrep=cgrep.astype(h16),
        ones_row=np.ones(BLK, h16),
        prelu_a=a,
    )


def _build_program():
    import concourse.bass as bass
    import concourse.bacc as bacc
    import concourse.mybir as mybir
    import concourse.tile as tile

    dt = mybir.dt
    f32 = dt.float32
    h16 = dt.float16
    AF = mybir.ActivationFunctionType
    ALU = mybir.AluOpType

    nc = bacc.Bacc("TRN2", target_bir_lowering=False, debug=False,
                   num_devices=B)

    xb = nc.dram_tensor("xb", [N, T], h16, kind="ExternalInput")
    mb = nc.dram_tensor("mb", [N, T], h16, kind="ExternalInput")
    adj = nc.dram_tensor("adj", [N, N], h16, kind="ExternalInput")
    cgrep = nc.dram_tensor("cgrep", [NT], h16, kind="ExternalInput")
    onesr = nc.dram_tensor("ones_row", [BLK], h16, kind="ExternalInput")
    zr_d = nc.dram_tensor("zr", [122, BLK], h16, kind="ExternalInput")
    pa_d = nc.dram_tensor("pa", [128, H], h16, kind="ExternalInput")
    pb_d = nc.dram_tensor("pb", [128, 128], h16, kind="ExternalInput")
    pc_d = nc.dram_tensor("pc", [128, 32], h16, kind="ExternalInput")
    bk_d = nc.dram_tensor("bk", [128, 1], f32, kind="ExternalInput")
    kk_d = nc.dram_tensor("kk", [128, 1], f32, kind="ExternalInput")
    bfs_d = nc.dram_tensor("bfs", [128, 1], f32, kind="ExternalInput")
    sc_d = nc.dram_tensor("sc", [128, 1], f32, kind="ExternalInput")
    out_d = nc.dram_tensor("out", [NT], f32, kind="ExternalOutput")

    MOVA_BUFS = 4
    RR_BUFS = 8
    from contextlib import ExitStack
    with tile.TileContext(nc) as tc, ExitStack() as ctx:
        const = ctx.enter_context(tc.tile_pool(name="const", bufs=1))
        adjp = ctx.enter_context(tc.tile_pool(name="adjp", bufs=1))
        gmovp = ctx.enter_context(tc.tile_pool(name="gmovp", bufs=1))
        movap = ctx.enter_context(tc.tile_pool(name="movap", bufs=1))
        rrp = ctx.enter_context(tc.tile_pool(name="rrp", bufs=RR_BUFS))
        posbp = ctx.enter_context(tc.tile_pool(name="posbp", bufs=3))
        psp = ctx.enter_context(tc.tile_pool(name="psp", bufs=6, space="PSUM"))
        pop = ctx.enter_context(tc.tile_pool(name="pop", bufs=2, space="PSUM"))

        pa_t = const.tile([128, H], h16)
        pb_t = const.tile([128, 128], h16)
        pc_t = const.tile([128, 32], h16)
        bk_t = const.tile([128, 1], f32)
        kk_t = const.tile([128, 1], f32)
        bfs_t = const.tile([128, 1], f32)
        sc_t = const.tile([128, 1], f32)
        nc.sync.dma_start(out=pa_t[:], in_=pa_d[:])
        nc.sync.dma_start(out=pb_t[:], in_=pb_d[:])
        nc.sync.dma_start(out=pc_t[:], in_=pc_d[:])
        nc.sync.dma_start(out=bk_t[:], in_=bk_d[:])
        nc.sync.dma_start(out=kk_t[:], in_=kk_d[:])
        nc.sync.dma_start(out=bfs_t[:], in_=bfs_d[:])
        nc.sync.dma_start(out=sc_t[:], in_=sc_d[:])

        # ---- x1/m streams (fp16) + adj tiles --------------------------
        gmov = []
        for nt in range(8):
            g = gmovp.tile([128, 128], h16, tag=f"gmov{nt}", name=f"gmov{nt}")
            nc.sync.dma_start(out=g[:, 0:64], in_=xb[nt * 128:(nt + 1) * 128, :])
            nc.sync.dma_start(out=g[:, 64:128], in_=mb[nt * 128:(nt + 1) * 128, :])
            nc.vector.scalar_tensor_tensor(
                out=g[:, 0:64], in0=g[:, 0:64], scalar=bfs_t[:, 0:1],
                in1=g[:, 64:128], op0=ALU.subtract, op1=ALU.mult)
            nc.vector.tensor_scalar_add(g[:, 0:64], g[:, 0:64], bfs_t[:, 0:1])
            gmov.append(g)
        adjt = []
        for nt in range(8):
            at = adjp.tile([128, 1024], h16, tag=f"adjt{nt}", name=f"adjt{nt}")
            nc.sync.dma_start(out=at[:], in_=adj[nt * 128:(nt + 1) * 128, :])
            adjt.append(at)

        # ---- software-pipelined G + tail over all 128 chunks ----------
        # ma2 layout: rows 0:6 streams, rows 6:64 zeros, rows 64:128 t1.
        # All matmuls K=128 (zero-padded stationaries) so the PE clock
        # monitor sees full-array activity and un-throttles to 2.4 GHz.
        # ma tiles are persistent (4, cycled per block): their zero rows
        # are initialized once; t1 rows are rewritten by ACT every chunk.
        ma4 = []
        for i in range(MOVA_BUFS):
            mai = movap.tile([128, BLK], h16, tag=f"mova{i}", name=f"mova{i}")
            nc.gpsimd.memset(mai[:, :], 0.0)
            nc.sync.dma_start(out=mai[5:6, :], in_=onesr[:])
            ma4.append(mai)
        gx = [None] * 8
        ma_t = [None] * NBLK
        pss = {}
        rrs = {}
        po_ps = {}

        def emit_g(mt):
            psg = psp.tile([128, 512], f32, tag="ps", name=f"psg{mt}")
            for nt in range(8):
                nc.tensor.matmul(
                    psg[:, 0:128],
                    adjt[nt][:, mt * 128:(mt + 1) * 128],
                    gmov[nt][:],
                    start=(nt == 0), stop=(nt == 7))
            g = gmovp.tile([128, 128], h16, tag=f"gxm{mt}", name=f"gxm{mt}")
            nc.vector.tensor_copy(g[:], psg[:, 0:128])
            gx[mt] = g

        def emit_ma(blk):
            mt, half = blk // 2, blk % 2
            p0 = half * 64
            ma = ma4[blk % MOVA_BUFS]
            nc.sync.dma_start(out=ma[0:1, :], in_=gmov[mt][p0:p0 + 64, 0:64])
            nc.sync.dma_start(out=ma[1:2, :], in_=gmov[mt][p0:p0 + 64, 64:128])
            nc.gpsimd.dma_start(out=ma[2:3, :], in_=gx[mt][p0:p0 + 64, 0:64])
            nc.gpsimd.dma_start(out=ma[3:4, :], in_=gx[mt][p0:p0 + 64, 64:128])
            nc.gpsimd.dma_start(out=ma[4:5, :],
                                in_=cgrep[blk * BLK:(blk + 1) * BLK])
            ma_t[blk] = ma

        def st_a(c):
            blk, j = c // 8, c % 8
            if c == 0:
                emit_g(0)
                emit_ma(0)
            if j == 0 and blk + 1 < NBLK:
                if (blk + 1) % 2 == 0:
                    emit_g((blk + 1) // 2)
                emit_ma(blk + 1)
            c0 = j * CHUNK
            # v at psum partitions 64:128 (fp16 col-shift), K=128
            ps_a = psp.tile([128, 512], f32, tag="ps", name=f"pa{c}")
            nc.tensor.matmul(ps_a[64:128, :], pa_t[:],
                             ma_t[blk][:, c0:c0 + CHUNK],
                             start=True, stop=True, tile_position=(0, 64))
            # t1 = (1-a)*relu(v), written back into ma2 rows 64:128
            nc.scalar.activation(ma_t[blk][64:128, c0:c0 + CHUNK],
                                 ps_a[64:128, :], AF.Relu,
                                 bias=0.0, scale=sc_t[64:128, 0:1])
            pss[c] = ps_a

        def st_b(c):
            blk, j = c // 8, c % 8
            c0 = j * CHUNK
            del pss[c]
            ps_v = psp.tile([128, 512], f32, tag="ps", name=f"pv{c}")
            nc.tensor.matmul(ps_v[:, :], pb_t[:],
                             ma_t[blk][:, c0:c0 + CHUNK],
                             start=True, stop=True)
            rr = rrp.tile([128, CHUNK], h16, tag="rr", name=f"rr{c}")
            nc.vector.tensor_scalar(
                out=rr[:, :], in0=ps_v[:, :],
                scalar1=bk_t[:, 0:1], scalar2=kk_t[:, 0:1],
                op0=ALU.add, op1=ALU.max)
            rrs[c] = rr

        def st_c(c):
            q = c % 4
            if q == 0:
                po_ps[c // 4] = pop.tile([128, 512], f32, tag="po",
                                         name=f"po{c // 4}")
            nc.tensor.matmul(po_ps[c // 4][32 * q:32 * q + 32, :], pc_t[:, :],
                             rrs[c][:, :], start=True, stop=True,
                             tile_position=(0, 32 * q))
            del rrs[c]
            if q == 3:
                po_sb = posbp.tile([97, 512], f32, tag="po_sb",
                                   name=f"po_sb{c // 4}")
                nc.scalar.activation(po_sb[0:97, :], po_ps[c // 4][0:97, :],
                                     AF.Identity, bias=0.0, scale=1.0)
                o0 = (c - 3) * CHUNK
                nc.sync.dma_start(out=out_d[o0:o0 + 4 * CHUNK],
                                  in_=po_sb[0:97:32, :])
                del po_ps[c // 4]

        for p in range(NCHUNK // 2 + 4):
            c = 2 * p
            if c < NCHUNK:
                st_a(c)
                st_a(c + 1)
            if 4 <= c < NCHUNK + 4:
                st_b(c - 4)
                st_b(c - 3)
            if c >= 8:
                st_c(c - 8)
                st_c(c - 7)

    nc.compile()
    return nc



def _get_program():
    if "prog" not in _CACHE:
        _CACHE["prog"] = _build_program()
    return _CACHE["prog"]


def kernel(x, mask, W_fs, b_fs, W_in, b_in, adj, W_gc, b_gc, W_lo, b_lo,
           prelu_a, W_ro, b_ro, W_o1, b_o1, W_o2, b_o2):
    x = np.asarray(x, np.float32)
    mask_f = np.asarray(mask, np.float16)
    adj = np.asarray(adj, np.float32)

    folded = _fold_weights(np.asarray(W_fs), np.asarray(b_fs),
                           np.asarray(W_in), np.asarray(b_in),
                           np.asarray(W_gc), np.asarray(b_gc),
                           np.asarray(W_lo), np.asarray(b_lo),
                           float(prelu_a),
                           np.asarray(W_ro), np.asarray(b_ro),
                           np.asarray(W_o1), np.asarray(b_o1),
                           np.asarray(W_o2), np.asarray(b_o2), adj)

    nc = _get_program()

    shared = dict(adj=adj.astype(np.float16), cgrep=folded["cgrep"],
                  ones_row=folded["ones_row"], zr=folded["zr"],
                  pa=folded["pa"], pb=folded["pb"], pc=folded["pc"],
                  bk=folded["bk"], kk=folded["kk"], bfs=folded["bfs"],
                  sc=folded["sc"])
    in_maps = []
    for b in range(B):
        m = dict(shared)
        m["xb"] = np.ascontiguousarray(x[b, 0]).astype(np.float16)
        m["mb"] = np.ascontiguousarray(mask_f[b, 0])
        in_maps.append(m)

    from concourse.bass_utils import run_bass_kernel_spmd
    res = run_bass_kernel_spmd(nc, in_maps, list(range(B)))

    out = np.empty((B, C, N, T), np.float32)
    for b in range(B):
        out[b, 0] = np.asarray(res.results[b]["out"]).reshape(N, T)
    return out



# revision 5
# speedup vs baseline: 1.1030x; 1.1030x over previous
"""Bass/Trainium2 kernel for nn_BiGRIL (gnn_message_passing).

Algebra (h == 0, C == 1 make the network collapse):
  x1  = where(mask, x, b_fs)
  v   = PA^T . streams            streams = [x1, m, xg, mg, cg, 1]
        with xg = A^T x1, mg = A^T m, cg = A^T 1   (graph diffusion)
  o   = PReLU(v) = max(a*v, v)    (ACT Lrelu, one op)
  f   = PB^T o                    PB = outer(w_ro1, W_o1[:,0])  (rank-1)
  rr  = relu(f + bk)              bk = W_o1[:,0]*b_ro + b_o1    (DVE, one op)
  out = W_o2 . rr  (+ b_o2 added on host)

Layout: t-major columns, col = t*N + n.  Sharding: data-parallel over
batch (B=8 -> 8 cores), weights + adj replicated, no collectives.

PE mapping: all streaming matmuls are K=64/M=64 in 64x64 array-tiling
mode (4 independent sub-arrays), so stages A, B, C of different chunks
run concurrently on disjoint tiles:
  A: (64,0)/(64,64)   B: (0,0)/(64,64)   C: (0,64)/(64,0)
G-phase keeps adj as the *moving* operand (8 LDWEIGHTS total).
Output rows accumulate into one PSUM bank via 16 column-shifted W_o2
stationaries (start=False), evacuated once per 32 chunks.
"""

import numpy as np
import sys

sys.path.insert(0, "/opt/trn_rl_repo")

B, C, N, T = 8, 1, 1024, 64
H = 64
NT = N * T            # 65536 columns per core, col = t*N + n
CHUNK = 512
NCHUNK = NT // CHUNK  # 128
GRP = 4               # chunks per group (one [128,1024] psum X-tile)
NGRP = NCHUNK // GRP  # 32
BLK = 4096            # ma-tile columns = 4 t-steps x 1024 nodes
NBLK = NT // BLK      # 16
NPC = 16              # column-shifted W_o2 variants (pairs per supergroup)

_CACHE = {}


def _fold_weights(W_fs, b_fs, W_in, b_in, W_gc, b_gc, W_lo, b_lo, prelu_a,
                  W_ro, b_ro, W_o1, b_o1, W_o2, b_o2, adj):
    f8 = np.float64
    W_in, b_in = W_in.astype(f8), b_in.astype(f8)
    W_gc, b_gc = W_gc.astype(f8), b_gc.astype(f8)
    W_lo, b_lo = W_lo.astype(f8), b_lo.astype(f8)
    W_ro, b_ro = W_ro.astype(f8), b_ro.astype(f8)
    W_o1, b_o1 = W_o1.astype(f8), b_o1.astype(f8)
    W_o2, b_o2 = W_o2.astype(f8), b_o2.astype(f8)

    W0 = W_in[:, 0]
    W1 = W_in[:, 1]
    Wlo1 = W_lo[:, :H]
    M1 = Wlo1 @ W_gc[:, :H]
    M2 = Wlo1 @ W_gc[:, H:]
    b_fold = Wlo1 @ b_gc + b_lo

    PA6 = np.stack([
        M1 @ W0, M1 @ W1, M2 @ W0, M2 @ W1,
        M2 @ b_in, M1 @ b_in + b_fold,
    ])                                      # [6, 64]

    w_ro1 = W_ro[0, :H]
    W_o1c = W_o1[:, 0]
    PB = np.outer(w_ro1, W_o1c)             # [64(K=o), 64(M=f)]
    bk = W_o1c * b_ro[0] + b_o1             # [64]

    h16, fp = np.float16, np.float32
    pa128 = np.zeros((128, H))
    pa128[64:70, :] = PA6
    pb128 = np.zeros((128, H))
    pb128[0:64, :] = PB
    pb128[64:128, :] = PB
    pc128 = np.zeros((128, NPC * H))
    for j in range(NPC):
        pc128[0:64, H * j + j] = W_o2[0]
        pc128[64:128, H * j + j] = W_o2[0]
    bk2 = np.zeros((128, 1))
    bk2[0:64, 0] = bk
    bk2[64:128, 0] = bk

    cg = adj.astype(f8).sum(axis=0)         # [N] col sums of adj
    cgt = np.tile(cg, T)                    # t-major: col = t*N + n

    return dict(
        pa=pa128.astype(h16), pb=pb128.astype(h16), pc=pc128.astype(h16),
        bk2=bk2.astype(fp), cgt=cgt.astype(h16),
        ones=np.ones(BLK, h16),
        bfs=float(b_fs[0]), a=float(prelu_a), b_o2=float(b_o2[0]),
    )


def _build_program(a_slope, bfs_val):
    import concourse.bass as bass
    import concourse.bacc as bacc
    import concourse.mybir as mybir
    import concourse.tile as tile

    dt = mybir.dt
    f32 = dt.float32
    h16 = dt.float16
    AF = mybir.ActivationFunctionType
    ALU = mybir.AluOpType

    nc = bacc.Bacc("TRN2", target_bir_lowering=False, debug=False,
                   num_devices=B)

    xT_d = nc.dram_tensor("xT", [T, N], h16, kind="ExternalInput")
    mT_d = nc.dram_tensor("mT", [T, N], h16, kind="ExternalInput")
    xn_d = nc.dram_tensor("xn", [128, 512], h16, kind="ExternalInput")
    mn_d = nc.dram_tensor("mn", [128, 512], h16, kind="ExternalInput")
    adj_d = nc.dram_tensor("adj", [N, N], h16, kind="ExternalInput")
    cgt_d = nc.dram_tensor("cgt", [NT], h16, kind="ExternalInput")
    ones_d = nc.dram_tensor("ones", [BLK], h16, kind="ExternalInput")
    pa_d = nc.dram_tensor("pa", [128, H], h16, kind="ExternalInput")
    pb_d = nc.dram_tensor("pb", [128, H], h16, kind="ExternalInput")
    pc_d = nc.dram_tensor("pc", [128, NPC * H], h16, kind="ExternalInput")
    bk2_d = nc.dram_tensor("bk2", [128, 1], f32, kind="ExternalInput")
    out_d = nc.dram_tensor("out", [NCHUNK, CHUNK], f32, kind="ExternalOutput")

    from contextlib import ExitStack
    with tile.TileContext(nc) as tc, ExitStack() as ctx:
        const = ctx.enter_context(tc.tile_pool(name="const", bufs=1))
        adjp = ctx.enter_context(tc.tile_pool(name="adjp", bufs=1))
        movap = ctx.enter_context(tc.tile_pool(name="movap", bufs=1))
        ttp = ctx.enter_context(tc.tile_pool(name="ttp", bufs=3))
        rtp = ctx.enter_context(tc.tile_pool(name="rtp", bufs=3))
        osp = ctx.enter_context(tc.tile_pool(name="osp", bufs=2))
        Xp = ctx.enter_context(tc.tile_pool(name="Xp", bufs=2, space="PSUM"))
        gp = ctx.enter_context(tc.tile_pool(name="gp", bufs=2, space="PSUM"))

        pa_t = const.tile([128, H], h16)
        pb_t = const.tile([128, H], h16)
        pc_t = const.tile([128, NPC * H], h16)
        bk2_t = const.tile([128, 1], f32)
        nc.sync.dma_start(out=pa_t[:], in_=pa_d[:])
        nc.sync.dma_start(out=pb_t[:], in_=pb_d[:])
        nc.sync.dma_start(out=pc_t[:], in_=pc_d[:])
        nc.sync.dma_start(out=bk2_t[:], in_=bk2_d[:])

        xT_t = const.tile([T, N], h16)
        mT_t = const.tile([T, N], h16)
        x1T_t = const.tile([T, N], h16)
        xn_t = const.tile([128, 512], h16)
        mn_t = const.tile([128, 512], h16)
        x1n_t = const.tile([128, 512], h16)
        gall = const.tile([128, 1024], h16)
        gxT = const.tile([128, 1024], h16)
        nc.sync.dma_start(out=xT_t[:], in_=xT_d[:])
        nc.sync.dma_start(out=mT_t[:], in_=mT_d[:])
        nc.sync.dma_start(out=xn_t[:], in_=xn_d[:])
        nc.sync.dma_start(out=mn_t[:], in_=mn_d[:])

        # x1 = (x - bfs)*m + bfs  in both orientations
        nc.vector.scalar_tensor_tensor(
            out=x1T_t[:], in0=xT_t[:], scalar=bfs_val, in1=mT_t[:],
            op0=ALU.subtract, op1=ALU.mult)
        nc.vector.tensor_scalar_add(x1T_t[:], x1T_t[:], bfs_val)
        nc.vector.scalar_tensor_tensor(
            out=x1n_t[:], in0=xn_t[:], scalar=bfs_val, in1=mn_t[:],
            op0=ALU.subtract, op1=ALU.mult)
        nc.vector.tensor_scalar_add(x1n_t[:], x1n_t[:], bfs_val)

        # gall: G stationary [128 n-parts, (x1 64t | m 64t)] per node-group
        for nt in range(8):
            nc.gpsimd.dma_start(out=gall[:, 128 * nt:128 * nt + 64],
                                in_=x1n_t[:, 64 * nt:64 * nt + 64])
            nc.gpsimd.dma_start(out=gall[:, 128 * nt + 64:128 * nt + 128],
                                in_=mn_t[:, 64 * nt:64 * nt + 64])

        adjt = []
        for nt in range(8):
            at = adjp.tile([128, 1024], h16, tag=f"adjt{nt}",
                           name=f"adjt{nt}")
            nc.sync.dma_start(out=at[:], in_=adj_d[nt * 128:(nt + 1) * 128, :])
            adjt.append(at)

        # ---- G: Xg[t | 64+t, m] = sum_n [x1|m][n,t] * adj[n,m] ----------
        Xg = Xp.tile([128, 1024], f32, tag="X", name="Xg")
        for nt in range(8):
            for hf in range(2):
                nc.tensor.matmul(
                    Xg[:, 512 * hf:512 * hf + 512],
                    gall[:, 128 * nt:128 * nt + 128],
                    adjt[nt][:, 512 * hf:512 * hf + 512],
                    start=(nt == 0), stop=(nt == 7))
        nc.vector.tensor_copy(gxT[:], Xg[:])

        # ---- persistent ma tiles: stream rows at partitions 64:70 -------
        # 64:x1 65:m 66:xg 67:mg 68:cg 69:ones; 70:128 zeroed (NaN guard)
        ma4 = []
        for i in range(4):
            mai = movap.tile([128, BLK], h16, tag=f"ma{i}", name=f"ma{i}")
            nc.gpsimd.memset(mai[64:128, :], 0.0)
            nc.sync.dma_start(out=mai[69:70, :], in_=ones_d[:])
            ma4.append(mai)

        def emit_ma(b):
            mat = ma4[b % 4]
            t0 = 4 * b
            nc.sync.dma_start(out=mat[64:65, :], in_=x1T_t[t0:t0 + 4, :])
            nc.sync.dma_start(out=mat[65:66, :], in_=mT_t[t0:t0 + 4, :])
            nc.gpsimd.dma_start(out=mat[66:67, :], in_=gxT[t0:t0 + 4, :])
            nc.gpsimd.dma_start(out=mat[67:68, :],
                                in_=gxT[64 + t0:64 + t0 + 4, :])
            nc.sync.dma_start(out=mat[68:69, :],
                              in_=cgt_d[BLK * b:BLK * (b + 1)])

        emit_ma(0)
        emit_ma(1)

        # ---- streaming pipeline -----------------------------------------
        Xs = {}
        tts = {}
        rts = {}
        gcur = {}

        def st_A(g):
            X = Xp.tile([128, 1024], f32, tag="X", name=f"X{g}")
            mat = ma4[(g // 2) % 4]
            for k in range(GRP):
                c = GRP * g + k
                wc = (c % 8) * CHUNK
                lo = (k % 2) * 64
                hf = (k // 2) * 512
                nc.tensor.matmul(
                    X[lo:lo + 64, hf:hf + 512],
                    pa_t[64:128, :],
                    mat[64:128, wc:wc + CHUNK],
                    start=True, stop=True, tile_position=(64, lo))
            Xs[g] = X

        def st_t(g):
            tt = ttp.tile([128, 1024], h16, tag="tt", name=f"tt{g}")
            nc.scalar.activation(tt[:], Xs[g][:], AF.Prelu,
                                 bias=0.0, scale=1.0, alpha=a_slope)
            tts[g] = tt

        def st_B(g):
            X, tt = Xs[g], tts[g]
            for k in range(GRP):
                lo = (k % 2) * 64
                hf = (k // 2) * 512
                nc.tensor.matmul(
                    X[lo:lo + 64, hf:hf + 512],
                    pb_t[lo:lo + 64, :],
                    tt[lo:lo + 64, hf:hf + 512],
                    start=True, stop=True, tile_position=(lo, lo))
            del tts[g]

        def st_rr(g):
            rt = rtp.tile([128, 1024], h16, tag="rt", name=f"rt{g}")
            nc.vector.tensor_scalar(
                out=rt[:], in0=Xs[g][:],
                scalar1=bk2_t[:, 0:1], scalar2=0.0,
                op0=ALU.add, op1=ALU.max)
            rts[g] = rt
            del Xs[g]

        def st_C(g):
            rt = rts[g]
            sg = g // 8
            if g % 8 == 0:
                gcur["e"] = gp.tile([128, 512], f32, tag="ge", name=f"ge{sg}")
                gcur["o"] = gp.tile([128, 512], f32, tag="go", name=f"go{sg}")
            ge, go = gcur["e"], gcur["o"]
            for k in range(GRP):
                p = 2 * g + k // 2
                j = p % NPC
                lo = (k % 2) * 64
                hf = (k // 2) * 512
                st = (j == 0)
                sp = (j == NPC - 1)
                if k % 2 == 0:
                    nc.tensor.matmul(
                        ge[64:128, :], pc_t[0:64, H * j:H * j + H],
                        rt[0:64, hf:hf + 512], start=st, stop=sp,
                        tile_position=(0, 64), skip_group_check=True)
                else:
                    nc.tensor.matmul(
                        go[0:64, :], pc_t[64:128, H * j:H * j + H],
                        rt[64:128, hf:hf + 512], start=st, stop=sp,
                        tile_position=(64, 0), skip_group_check=True)
            del rts[g]
            if g % 8 == 7:
                osb = osp.tile([128, 512], f32, tag="os", name=f"os{sg}")
                nc.scalar.copy(osb[64:80, :], ge[64:80, :])
                nc.scalar.copy(osb[0:16, :], go[0:16, :])
                nc.sync.dma_start(out=out_d[32 * sg:32 * sg + 32:2, :],
                                  in_=osb[64:80, :])
                nc.sync.dma_start(out=out_d[32 * sg + 1:32 * sg + 32:2, :],
                                  in_=osb[0:16, :])

        for g in range(NGRP + 2):
            if g < NGRP:
                if g % 2 == 0 and g // 2 + 2 < NBLK:
                    emit_ma(g // 2 + 2)
                st_A(g)
                st_t(g)
            if 1 <= g < NGRP + 1:
                st_B(g - 1)
                st_rr(g - 1)
            if g >= 2:
                st_C(g - 2)

    nc.compile()
    return nc


def _get_program(a_slope, bfs_val):
    if "prog" not in _CACHE:
        _CACHE["prog"] = _build_program(a_slope, bfs_val)
    return _CACHE["prog"]


def _make_in_maps(x, mask_f, adj16, folded):
    shared = dict(adj=adj16, cgt=folded["cgt"], ones=folded["ones"],
                  pa=folded["pa"], pb=folded["pb"], pc=folded["pc"],
                  bk2=folded["bk2"])
    in_maps = []
    for b in range(B):
        xb = np.ascontiguousarray(x[b, 0]).astype(np.float16)   # [N, T]
        mb = np.ascontiguousarray(mask_f[b, 0])                 # [N, T] f16
        m = dict(shared)
        m["xT"] = np.ascontiguousarray(xb.T)
        m["mT"] = np.ascontiguousarray(mb.T)
        m["xn"] = np.ascontiguousarray(
            xb.reshape(8, 128, T).transpose(1, 0, 2).reshape(128, 512))
        m["mn"] = np.ascontiguousarray(
            mb.reshape(8, 128, T).transpose(1, 0, 2).reshape(128, 512))
        in_maps.append(m)
    return in_maps


def kernel(x, mask, W_fs, b_fs, W_in, b_in, adj, W_gc, b_gc, W_lo, b_lo,
           prelu_a, W_ro, b_ro, W_o1, b_o1, W_o2, b_o2):
    x = np.asarray(x, np.float32)
    mask_f = np.asarray(mask, np.float16)
    adj = np.asarray(adj, np.float32)

    folded = _fold_weights(np.asarray(W_fs), np.asarray(b_fs),
                           np.asarray(W_in), np.asarray(b_in),
                           np.asarray(W_gc), np.asarray(b_gc),
                           np.asarray(W_lo), np.asarray(b_lo),
                           float(prelu_a),
                           np.asarray(W_ro), np.asarray(b_ro),
                           np.asarray(W_o1), np.asarray(b_o1),
                           np.asarray(W_o2), np.asarray(b_o2), adj)

    nc = _get_program(folded["a"], folded["bfs"])
    in_maps = _make_in_maps(x, mask_f, adj.astype(np.float16), folded)

    from concourse.bass_utils import run_bass_kernel_spmd
    res = run_bass_kernel_spmd(nc, in_maps, list(range(B)))

    out = np.empty((B, C, N, T), np.float32)
    for b in range(B):
        ob = np.asarray(res.results[b]["out"]).reshape(T, N)
        out[b, 0] = ob.T + folded["b_o2"]
    return out


# revision 9
# speedup vs baseline: 1.2276x; 1.1130x over previous
"""Bass/Trainium2 kernel for nn_BiGRIL (gnn_message_passing).

Algebra (h == 0, C == 1 make the network collapse):
  x1  = where(mask, x, b_fs)
  v   = PA^T . streams            streams = [x1, m, xg, mg, cg, 1]
        with xg = A^T x1, mg = A^T m, cg = A^T 1   (graph diffusion)
  o   = PReLU(v) = max(a*v, v)    (ACT Prelu, one op per 4 chunks)
  f   = PB^T o                    PB = outer(w_ro1, W_o1[:,0])  (rank-1)
  rr  = relu(f + bk)              bk = W_o1[:,0]*b_ro + b_o1    (DVE, one op)
  out = W_o2 . rr  (+ b_o2 added on host)

Layout: t-major columns, col = t*N + n.  Sharding: data-parallel over
batch (B=8 -> 8 cores), weights + adj replicated, no collectives.

PE mapping: every matmul is full-array K=128/M=128/N=512 (no array
tiling, no mode switches -> the HAM clock gate stays at 8/8 = 2.4 GHz).
Each pass packs TWO 512-col chunks block-diagonally into K:
  A: lhsT rows 0:6  -> cols 0:64  (even chunk streams -> v)
     lhsT rows 64:70-> cols 64:128 (odd chunk streams -> v)
  B: [PB 0; 0 PB], C: col 2j+s = W_o2 on rows 64s:64s+64
so each of A/B/C streams only NT/2 columns.  C accumulates 32 pairs
(64 chunks) of output rows into one PSUM bank via column-shifted
stationaries (start=False); 2 evacuations per kernel.
G-phase keeps adj as the *moving* operand (8 LDWEIGHTS total).
"""

import numpy as np
import sys

sys.path.insert(0, "/opt/trn_rl_repo")

B, C, N, T = 8, 1, 1024, 64
H = 64
NT = N * T            # 65536 columns per core, col = t*N + n
CHUNK = 512
NPAIR = 64            # pair p = t-step p: chunks (2p, 2p+1) = n-halves
NGRP = 32             # grp g = pairs (2g, 2g+1) -> one [128,1024] X tile
BLK = 2048            # ma-tile columns = 4 pairs (4 t-steps)
NBLK = 16
NSC = 32              # column-shifted W_o2 variants (pairs per out bank)

_CACHE = {}


def _fold_weights(W_fs, b_fs, W_in, b_in, W_gc, b_gc, W_lo, b_lo, prelu_a,
                  W_ro, b_ro, W_o1, b_o1, W_o2, b_o2, adj):
    f8 = np.float64
    W_in, b_in = W_in.astype(f8), b_in.astype(f8)
    W_gc, b_gc = W_gc.astype(f8), b_gc.astype(f8)
    W_lo, b_lo = W_lo.astype(f8), b_lo.astype(f8)
    W_ro, b_ro = W_ro.astype(f8), b_ro.astype(f8)
    W_o1, b_o1 = W_o1.astype(f8), b_o1.astype(f8)
    W_o2, b_o2 = W_o2.astype(f8), b_o2.astype(f8)

    W0 = W_in[:, 0]
    W1 = W_in[:, 1]
    Wlo1 = W_lo[:, :H]
    M1 = Wlo1 @ W_gc[:, :H]
    M2 = Wlo1 @ W_gc[:, H:]
    b_fold = Wlo1 @ b_gc + b_lo

    PA6 = np.stack([
        M1 @ W0, M1 @ W1, M2 @ W0, M2 @ W1,
        M2 @ b_in, M1 @ b_in + b_fold,
    ])                                      # [6, 64]

    w_ro1 = W_ro[0, :H]
    W_o1c = W_o1[:, 0]
    PB = np.outer(w_ro1, W_o1c)             # [64(K=o), 64(M=f)]
    bk = W_o1c * b_ro[0] + b_o1             # [64]

    h16, fp = np.float16, np.float32
    sa = np.zeros((128, 128))
    sa[0:6, 0:64] = PA6
    sa[64:70, 64:128] = PA6
    sb = np.zeros((128, 128))
    sb[0:64, 0:64] = PB
    sb[64:128, 64:128] = PB
    sc = np.zeros((128, NSC * 128))
    for j in range(NSC):
        sc[0:64, 128 * j + 2 * j] = W_o2[0]
        sc[64:128, 128 * j + 2 * j + 1] = W_o2[0]
    bk2 = np.zeros((128, 1))
    bk2[0:64, 0] = bk
    bk2[64:128, 0] = bk

    cg = adj.astype(f8).sum(axis=0)         # [N] col sums of adj
    cge = np.tile(cg[:512], 4)              # even-chunk cg row per ma block
    cgo = np.tile(cg[512:], 4)

    return dict(
        sa=sa.astype(h16), sb=sb.astype(h16), sc=sc.astype(h16),
        bk2=bk2.astype(fp), cge=cge.astype(h16), cgo=cgo.astype(h16),
        ones=np.ones(BLK, h16),
        bfs=float(b_fs[0]), a=float(prelu_a), b_o2=float(b_o2[0]),
    )


def _build_program(a_slope, bfs_val):
    import concourse.bass as bass
    import concourse.bacc as bacc
    import concourse.mybir as mybir
    import concourse.tile as tile

    dt = mybir.dt
    f32 = dt.float32
    h16 = dt.float16
    AF = mybir.ActivationFunctionType
    ALU = mybir.AluOpType

    nc = bacc.Bacc("TRN2", target_bir_lowering=False, debug=False,
                   num_devices=B)

    xT_d = nc.dram_tensor("xT", [T, N], h16, kind="ExternalInput")
    mT_d = nc.dram_tensor("mT", [T, N], h16, kind="ExternalInput")
    xn_d = nc.dram_tensor("xn", [128, 512], h16, kind="ExternalInput")
    mn_d = nc.dram_tensor("mn", [128, 512], h16, kind="ExternalInput")
    adj_d = nc.dram_tensor("adj", [N, N], h16, kind="ExternalInput")
    cge_d = nc.dram_tensor("cge", [BLK], h16, kind="ExternalInput")
    ones_d = nc.dram_tensor("ones", [BLK], h16, kind="ExternalInput")
    cgo_d = nc.dram_tensor("cgo", [BLK], h16, kind="ExternalInput")
    sa_d = nc.dram_tensor("sa", [128, 128], h16, kind="ExternalInput")
    sb_d = nc.dram_tensor("sb", [128, 128], h16, kind="ExternalInput")
    sc_d = nc.dram_tensor("sc", [128, NSC * 128], h16, kind="ExternalInput")
    bk2_d = nc.dram_tensor("bk2", [128, 1], f32, kind="ExternalInput")
    out_d = nc.dram_tensor("out", [2 * NPAIR, CHUNK], f32,
                           kind="ExternalOutput")

    from contextlib import ExitStack
    with tile.TileContext(nc) as tc, ExitStack() as ctx:
        const = ctx.enter_context(tc.tile_pool(name="const", bufs=1))
        adjp = ctx.enter_context(tc.tile_pool(name="adjp", bufs=1))
        movap = ctx.enter_context(tc.tile_pool(name="movap", bufs=1))
        ttp = ctx.enter_context(tc.tile_pool(name="ttp", bufs=3))
        rtp = ctx.enter_context(tc.tile_pool(name="rtp", bufs=3))
        osp = ctx.enter_context(tc.tile_pool(name="osp", bufs=2))
        Xp = ctx.enter_context(tc.tile_pool(name="Xp", bufs=3, space="PSUM"))
        gp = ctx.enter_context(tc.tile_pool(name="gp", bufs=2, space="PSUM"))

        sa_t = const.tile([128, 128], h16)
        sb_t = const.tile([128, 128], h16)
        sc_t = const.tile([128, NSC * 128], h16)
        bk2_t = const.tile([128, 1], f32)
        nc.sync.dma_start(out=sa_t[:], in_=sa_d[:])
        nc.sync.dma_start(out=sb_t[:], in_=sb_d[:])
        nc.sync.dma_start(out=sc_t[:], in_=sc_d[:])
        nc.sync.dma_start(out=bk2_t[:], in_=bk2_d[:])

        xT_t = const.tile([T, N], h16)
        mT_t = const.tile([T, N], h16)
        x1T_t = const.tile([T, N], h16)
        xn_t = const.tile([128, 512], h16)
        mn_t = const.tile([128, 512], h16)
        x1n_t = const.tile([128, 512], h16)
        gall = const.tile([128, 1024], h16)
        gxT = const.tile([128, 1024], h16)
        nc.sync.dma_start(out=xT_t[:], in_=xT_d[:])
        nc.sync.dma_start(out=mT_t[:], in_=mT_d[:])
        nc.sync.dma_start(out=xn_t[:], in_=xn_d[:])
        nc.sync.dma_start(out=mn_t[:], in_=mn_d[:])

        # x1 = (x - bfs)*m + bfs  in both orientations
        nc.vector.scalar_tensor_tensor(
            out=x1T_t[:], in0=xT_t[:], scalar=bfs_val, in1=mT_t[:],
            op0=ALU.subtract, op1=ALU.mult)
        nc.vector.tensor_scalar_add(x1T_t[:], x1T_t[:], bfs_val)
        nc.vector.scalar_tensor_tensor(
            out=x1n_t[:], in0=xn_t[:], scalar=bfs_val, in1=mn_t[:],
            op0=ALU.subtract, op1=ALU.mult)
        nc.vector.tensor_scalar_add(x1n_t[:], x1n_t[:], bfs_val)

        # gall: G stationary [128 n-parts, (x1 64t | m 64t)] per node-group
        for nt in range(8):
            nc.gpsimd.dma_start(out=gall[:, 128 * nt:128 * nt + 64],
                                in_=x1n_t[:, 64 * nt:64 * nt + 64])
            nc.gpsimd.dma_start(out=gall[:, 128 * nt + 64:128 * nt + 128],
                                in_=mn_t[:, 64 * nt:64 * nt + 64])

        adjt = []
        for nt in range(8):
            at = adjp.tile([128, 1024], h16, tag=f"adjt{nt}",
                           name=f"adjt{nt}")
            nc.sync.dma_start(out=at[:], in_=adj_d[nt * 128:(nt + 1) * 128, :])
            adjt.append(at)

        # ---- G: Xg[t | 64+t, m] = sum_n [x1|m][n,t] * adj[n,m] ----------
        Xg = Xp.tile([128, 1024], f32, tag="X", name="Xg")
        for nt in range(8):
            for hf in range(2):
                nc.tensor.matmul(
                    Xg[:, 512 * hf:512 * hf + 512],
                    gall[:, 128 * nt:128 * nt + 128],
                    adjt[nt][:, 512 * hf:512 * hf + 512],
                    start=(nt == 0), stop=(nt == 7))
        nc.vector.tensor_copy(gxT[:], Xg[:])

        # ---- persistent ma tiles: [128, 2048] = 4 pairs (t-steps) -------
        # rows 0:6  = even-chunk streams (n 0:512):  x1, m, xg, mg, cg, 1
        # rows 64:70= odd-chunk streams  (n 512:1024)
        ma4 = []
        for i in range(4):
            mai = movap.tile([128, BLK], h16, tag=f"ma{i}", name=f"ma{i}")
            nc.gpsimd.memset(mai[0:64, :], 0.0)
            nc.gpsimd.memset(mai[64:128, :], 0.0)
            nc.sync.dma_start(out=mai[5:6, :], in_=ones_d[:])
            nc.sync.dma_start(out=mai[69:70, :], in_=ones_d[:])
            nc.sync.dma_start(out=mai[4:5, :], in_=cge_d[:])
            nc.sync.dma_start(out=mai[68:69, :], in_=cgo_d[:])
            ma4.append(mai)

        def emit_ma(b):
            mat = ma4[b % 4]
            t0 = 4 * b
            nc.sync.dma_start(out=mat[0:1, :], in_=x1T_t[t0:t0 + 4, 0:512])
            nc.sync.dma_start(out=mat[64:65, :], in_=x1T_t[t0:t0 + 4, 512:1024])
            nc.sync.dma_start(out=mat[1:2, :], in_=mT_t[t0:t0 + 4, 0:512])
            nc.sync.dma_start(out=mat[65:66, :], in_=mT_t[t0:t0 + 4, 512:1024])
            nc.gpsimd.dma_start(out=mat[2:3, :], in_=gxT[t0:t0 + 4, 0:512])
            nc.gpsimd.dma_start(out=mat[66:67, :], in_=gxT[t0:t0 + 4, 512:1024])
            nc.gpsimd.dma_start(out=mat[3:4, :],
                                in_=gxT[64 + t0:64 + t0 + 4, 0:512])
            nc.gpsimd.dma_start(out=mat[67:68, :],
                                in_=gxT[64 + t0:64 + t0 + 4, 512:1024])

        emit_ma(0)
        emit_ma(1)

        # ---- streaming pipeline -----------------------------------------
        Xs = {}
        tts = {}
        rts = {}
        gcur = {}

        def st_A(g):
            X = Xp.tile([128, 1024], f32, tag="X", name=f"X{g}")
            for h in range(2):
                p = 2 * g + h
                mat = ma4[(p // 4) % 4]
                mc = (p % 4) * CHUNK
                nc.tensor.matmul(
                    X[:, 512 * h:512 * h + 512], sa_t[:],
                    mat[:, mc:mc + CHUNK], start=True, stop=True)
            Xs[g] = X

        def st_t(g):
            tt = ttp.tile([128, 1024], h16, tag="tt", name=f"tt{g}")
            nc.scalar.activation(tt[:], Xs[g][:], AF.Prelu,
                                 bias=0.0, scale=1.0, alpha=a_slope)
            tts[g] = tt

        def st_B(g):
            X, tt = Xs[g], tts[g]
            for h in range(2):
                nc.tensor.matmul(
                    X[:, 512 * h:512 * h + 512], sb_t[:],
                    tt[:, 512 * h:512 * h + 512], start=True, stop=True)
            del tts[g]

        def st_rr(g):
            rt = rtp.tile([128, 1024], h16, tag="rt", name=f"rt{g}")
            nc.vector.tensor_scalar(
                out=rt[:], in0=Xs[g][:],
                scalar1=bk2_t[:, 0:1], scalar2=0.0,
                op0=ALU.add, op1=ALU.max)
            rts[g] = rt
            del Xs[g]

        def st_C(g):
            rt = rts[g]
            for h in range(2):
                p = 2 * g + h
                s = p // NSC
                j = p % NSC
                if j == 0:
                    gcur[s] = gp.tile([128, 512], f32, tag="go",
                                      name=f"go{s}")
                gam = gcur[s]
                nc.tensor.matmul(
                    gam[:], sc_t[:, 128 * j:128 * j + 128],
                    rt[:, 512 * h:512 * h + 512],
                    start=(j == 0), stop=(j == NSC - 1),
                    skip_group_check=True)
                if j == NSC - 1:
                    osb = osp.tile([128, 512], f32, tag="os", name=f"os{s}")
                    nc.scalar.copy(osb[0:64, :], gam[0:64, :])
                    nc.sync.dma_start(out=out_d[64 * s:64 * s + 64, :],
                                      in_=osb[0:64, :])
                    del gcur[s]
            del rts[g]

        for g in range(NGRP + 2):
            if g < NGRP:
                if g % 2 == 0 and g // 2 + 2 < NBLK:
                    emit_ma(g // 2 + 2)
                st_A(g)
                st_t(g)
            if 1 <= g < NGRP + 1:
                st_B(g - 1)
                st_rr(g - 1)
            if g >= 2:
                st_C(g - 2)

    nc.compile()
    return nc


def _get_program(a_slope, bfs_val):
    if "prog" not in _CACHE:
        _CACHE["prog"] = _build_program(a_slope, bfs_val)
    return _CACHE["prog"]


def _make_in_maps(x, mask_f, adj16, folded):
    shared = dict(adj=adj16, cge=folded["cge"], cgo=folded["cgo"],
                  ones=folded["ones"],
                  sa=folded["sa"], sb=folded["sb"], sc=folded["sc"],
                  bk2=folded["bk2"])
    in_maps = []
    for b in range(B):
        xb = np.ascontiguousarray(x[b, 0]).astype(np.float16)   # [N, T]
        mb = np.ascontiguousarray(mask_f[b, 0])                 # [N, T] f16
        m = dict(shared)
        m["xT"] = np.ascontiguousarray(xb.T)
        m["mT"] = np.ascontiguousarray(mb.T)
        m["xn"] = np.ascontiguousarray(
            xb.reshape(8, 128, T).transpose(1, 0, 2).reshape(128, 512))
        m["mn"] = np.ascontiguousarray(
            mb.reshape(8, 128, T).transpose(1, 0, 2).reshape(128, 512))
        in_maps.append(m)
    return in_maps


def kernel(x, mask, W_fs, b_fs, W_in, b_in, adj, W_gc, b_gc, W_lo, b_lo,
           prelu_a, W_ro, b_ro, W_o1, b_o1, W_o2, b_o2):
    x = np.asarray(x, np.float32)
    mask_f = np.asarray(mask, np.float16)
    adj = np.asarray(adj, np.float32)

    folded = _fold_weights(np.asarray(W_fs), np.asarray(b_fs),
                           np.asarray(W_in), np.asarray(b_in),
                           np.asarray(W_gc), np.asarray(b_gc),
                           np.asarray(W_lo), np.asarray(b_lo),
                           float(prelu_a),
                           np.asarray(W_ro), np.asarray(b_ro),
                           np.asarray(W_o1), np.asarray(b_o1),
                           np.asarray(W_o2), np.asarray(b_o2), adj)

    nc = _get_program(folded["a"], folded["bfs"])
    in_maps = _make_in_maps(x, mask_f, adj.astype(np.float16), folded)

    from concourse.bass_utils import run_bass_kernel_spmd
    res = run_bass_kernel_spmd(nc, in_maps, list(range(B)))

    out = np.empty((B, C, N, T), np.float32)
    for b in range(B):
        # out row r = chunk r; chunk 2p+h = (t=p, n-half h)
        ob = np.asarray(res.results[b]["out"]).reshape(T, 2, 512)
        ob = ob.transpose(0, 1, 2).reshape(T, N)   # [t, n]
        out[b, 0] = ob.T + folded["b_o2"]
    return out


# revision 10
# speedup vs baseline: 1.2589x; 1.0256x over previous
"""Bass/Trainium2 kernel for nn_BiGRIL (gnn_message_passing).

Algebra (h == 0, C == 1 make the network collapse):
  x1  = where(mask, x, b_fs)
  v   = PA^T . streams            streams = [x1, m, xg, mg, cg, 1]
        with xg = A^T x1, mg = A^T m, cg = A^T 1   (graph diffusion)
  o   = PReLU(v) = max(a*v, v)    (ACT Prelu, one op per 4 chunks)
  f   = PB^T o                    PB = outer(w_ro1, W_o1[:,0])  (rank-1)
  rr  = relu(f + bk)              bk = W_o1[:,0]*b_ro + b_o1    (DVE, one op)
  out = W_o2 . rr  (+ b_o2 added on host)

Layout: t-major columns, col = t*N + n.  Sharding: data-parallel over
batch (B=8 -> 8 cores), weights + adj replicated, no collectives.

PE mapping: every matmul is full-array K=128/M=128/N=512 (no array
tiling, no mode switches -> the HAM clock gate stays at 8/8 = 2.4 GHz).
Each pass packs TWO 512-col chunks block-diagonally into K:
  A: lhsT rows 0:6  -> cols 0:64  (even chunk streams -> v)
     lhsT rows 64:70-> cols 64:128 (odd chunk streams -> v)
  B: [PB 0; 0 PB], C: col 2j+s = W_o2 on rows 64s:64s+64
so each of A/B/C streams only NT/2 columns.  C accumulates 32 pairs
(64 chunks) of output rows into one PSUM bank via column-shifted
stationaries (start=False); 2 evacuations per kernel.
G-phase keeps adj as the *moving* operand (8 LDWEIGHTS total).
"""

import numpy as np
import sys

sys.path.insert(0, "/opt/trn_rl_repo")

B, C, N, T = 8, 1, 1024, 64
H = 64
NT = N * T            # 65536 columns per core, col = t*N + n
CHUNK = 512
NPAIR = 64            # pair p = t-step p: chunks (2p, 2p+1) = n-halves
NGRP = 32             # grp g = pairs (2g, 2g+1) -> one [128,1024] X tile
BLK = 2048            # ma-tile columns = 4 pairs (4 t-steps)
NBLK = 16
NSC = 32              # column-shifted W_o2 variants (pairs per out bank)

_CACHE = {}


def _fold_weights(W_fs, b_fs, W_in, b_in, W_gc, b_gc, W_lo, b_lo, prelu_a,
                  W_ro, b_ro, W_o1, b_o1, W_o2, b_o2, adj):
    f8 = np.float64
    W_in, b_in = W_in.astype(f8), b_in.astype(f8)
    W_gc, b_gc = W_gc.astype(f8), b_gc.astype(f8)
    W_lo, b_lo = W_lo.astype(f8), b_lo.astype(f8)
    W_ro, b_ro = W_ro.astype(f8), b_ro.astype(f8)
    W_o1, b_o1 = W_o1.astype(f8), b_o1.astype(f8)
    W_o2, b_o2 = W_o2.astype(f8), b_o2.astype(f8)

    W0 = W_in[:, 0]
    W1 = W_in[:, 1]
    Wlo1 = W_lo[:, :H]
    M1 = Wlo1 @ W_gc[:, :H]
    M2 = Wlo1 @ W_gc[:, H:]
    b_fold = Wlo1 @ b_gc + b_lo

    PA6 = np.stack([
        M1 @ W0, M1 @ W1, M2 @ W0, M2 @ W1,
        M2 @ b_in, M1 @ b_in + b_fold,
    ])                                      # [6, 64]

    w_ro1 = W_ro[0, :H]
    W_o1c = W_o1[:, 0]
    PB = np.outer(w_ro1, W_o1c)             # [64(K=o), 64(M=f)]
    bk = W_o1c * b_ro[0] + b_o1             # [64]

    h16, fp = np.float16, np.float32
    sa = np.zeros((128, 128))
    sa[0:6, 0:64] = PA6
    sa[64:70, 64:128] = PA6
    sb = np.zeros((128, 128))
    sb[0:64, 0:64] = PB
    sb[64:128, 64:128] = PB
    sc = np.zeros((128, NSC * 128))
    for j in range(NSC):
        sc[0:64, 128 * j + 2 * j] = W_o2[0]
        sc[64:128, 128 * j + 2 * j + 1] = W_o2[0]
    bk2 = np.zeros((128, 1))
    bk2[0:64, 0] = bk
    bk2[64:128, 0] = bk

    cg = adj.astype(f8).sum(axis=0)         # [N] col sums of adj
    cge = np.tile(cg[:512], 4)              # even-chunk cg row per ma block
    cgo = np.tile(cg[512:], 4)

    return dict(
        sa=sa.astype(h16), sb=sb.astype(h16), sc=sc.astype(h16),
        bk2=bk2.astype(fp), cge=cge.astype(h16), cgo=cgo.astype(h16),
        ones=np.ones(BLK, h16),
        bfs=float(b_fs[0]), a=float(prelu_a), b_o2=float(b_o2[0]),
    )


def _build_program(a_slope, bfs_val):
    import concourse.bass as bass
    import concourse.bacc as bacc
    import concourse.mybir as mybir
    import concourse.tile as tile

    dt = mybir.dt
    f32 = dt.float32
    h16 = dt.float16
    AF = mybir.ActivationFunctionType
    ALU = mybir.AluOpType

    nc = bacc.Bacc("TRN2", target_bir_lowering=False, debug=False,
                   num_devices=B)

    xT_d = nc.dram_tensor("xT", [T, N], h16, kind="ExternalInput")
    mT_d = nc.dram_tensor("mT", [T, N], h16, kind="ExternalInput")
    xn_d = nc.dram_tensor("xn", [128, 512], h16, kind="ExternalInput")
    mn_d = nc.dram_tensor("mn", [128, 512], h16, kind="ExternalInput")
    adj_d = nc.dram_tensor("adj", [N, N], h16, kind="ExternalInput")
    cge_d = nc.dram_tensor("cge", [BLK], h16, kind="ExternalInput")
    ones_d = nc.dram_tensor("ones", [BLK], h16, kind="ExternalInput")
    cgo_d = nc.dram_tensor("cgo", [BLK], h16, kind="ExternalInput")
    sa_d = nc.dram_tensor("sa", [128, 128], h16, kind="ExternalInput")
    sb_d = nc.dram_tensor("sb", [128, 128], h16, kind="ExternalInput")
    sc_d = nc.dram_tensor("sc", [128, NSC * 128], h16, kind="ExternalInput")
    bk2_d = nc.dram_tensor("bk2", [128, 1], f32, kind="ExternalInput")
    out_d = nc.dram_tensor("out", [2 * NPAIR, CHUNK], f32,
                           kind="ExternalOutput")

    from contextlib import ExitStack
    with tile.TileContext(nc) as tc, ExitStack() as ctx:
        const = ctx.enter_context(tc.tile_pool(name="const", bufs=1))
        adjp = ctx.enter_context(tc.tile_pool(name="adjp", bufs=1))
        movap = ctx.enter_context(tc.tile_pool(name="movap", bufs=1))
        ttp = ctx.enter_context(tc.tile_pool(name="ttp", bufs=3))
        rtp = ctx.enter_context(tc.tile_pool(name="rtp", bufs=3))
        osp = ctx.enter_context(tc.tile_pool(name="osp", bufs=2))
        Xp = ctx.enter_context(tc.tile_pool(name="Xp", bufs=3, space="PSUM"))
        gp = ctx.enter_context(tc.tile_pool(name="gp", bufs=2, space="PSUM"))

        sa_t = const.tile([128, 128], h16)
        sb_t = const.tile([128, 128], h16)
        sc_t = const.tile([128, NSC * 128], h16)
        bk2_t = const.tile([128, 1], f32)
        nc.sync.dma_start(out=sa_t[:], in_=sa_d[:])
        nc.sync.dma_start(out=sb_t[:], in_=sb_d[:])
        nc.sync.dma_start(out=sc_t[:], in_=sc_d[:])
        nc.sync.dma_start(out=bk2_t[:], in_=bk2_d[:])

        xT_t = const.tile([T, N], h16)
        mT_t = const.tile([T, N], h16)
        x1T_t = const.tile([T, N], h16)
        xn_t = const.tile([128, 512], h16)
        mn_t = const.tile([128, 512], h16)
        x1n_t = const.tile([128, 512], h16)
        gall = const.tile([128, 1024], h16)
        gxT = const.tile([128, 1024], h16)
        nc.sync.dma_start(out=xT_t[:], in_=xT_d[:])
        nc.sync.dma_start(out=mT_t[:], in_=mT_d[:])
        nc.sync.dma_start(out=xn_t[:], in_=xn_d[:])
        nc.sync.dma_start(out=mn_t[:], in_=mn_d[:])

        # x1 = (x - bfs)*m + bfs  in both orientations
        nc.vector.scalar_tensor_tensor(
            out=x1T_t[:], in0=xT_t[:], scalar=bfs_val, in1=mT_t[:],
            op0=ALU.subtract, op1=ALU.mult)
        nc.vector.tensor_scalar_add(x1T_t[:], x1T_t[:], bfs_val)
        nc.vector.scalar_tensor_tensor(
            out=x1n_t[:], in0=xn_t[:], scalar=bfs_val, in1=mn_t[:],
            op0=ALU.subtract, op1=ALU.mult)
        nc.vector.tensor_scalar_add(x1n_t[:], x1n_t[:], bfs_val)

        # gall: G stationary [128 n-parts, (x1 64t | m 64t)] per node-group
        for nt in range(8):
            nc.gpsimd.dma_start(out=gall[:, 128 * nt:128 * nt + 64],
                                in_=x1n_t[:, 64 * nt:64 * nt + 64])
            nc.gpsimd.dma_start(out=gall[:, 128 * nt + 64:128 * nt + 128],
                                in_=mn_t[:, 64 * nt:64 * nt + 64])

        adjt = []
        for nt in range(8):
            at = adjp.tile([128, 1024], h16, tag=f"adjt{nt}",
                           name=f"adjt{nt}")
            nc.sync.dma_start(out=at[:], in_=adj_d[nt * 128:(nt + 1) * 128, :])
            adjt.append(at)

        # ---- G: Xg[t | 64+t, m] = sum_n [x1|m][n,t] * adj[n,m] ----------
        Xg = Xp.tile([128, 1024], f32, tag="X", name="Xg")
        for nt in range(8):
            for hf in range(2):
                nc.tensor.matmul(
                    Xg[:, 512 * hf:512 * hf + 512],
                    gall[:, 128 * nt:128 * nt + 128],
                    adjt[nt][:, 512 * hf:512 * hf + 512],
                    start=(nt == 0), stop=(nt == 7))
        nc.vector.tensor_copy(gxT[:], Xg[:])

        # ---- persistent ma tiles: [128, 2048] = 4 pairs (t-steps) -------
        # rows 0:6  = even-chunk streams (n 0:512):  x1, m, xg, mg, cg, 1
        # rows 64:70= odd-chunk streams  (n 512:1024)
        ma4 = []
        for i in range(4):
            mai = movap.tile([128, BLK], h16, tag=f"ma{i}", name=f"ma{i}")
            nc.gpsimd.memset(mai[0:64, :], 0.0)
            nc.gpsimd.memset(mai[64:128, :], 0.0)
            nc.sync.dma_start(out=mai[5:6, :], in_=ones_d[:])
            nc.sync.dma_start(out=mai[69:70, :], in_=ones_d[:])
            nc.sync.dma_start(out=mai[4:5, :], in_=cge_d[:])
            nc.sync.dma_start(out=mai[68:69, :], in_=cgo_d[:])
            ma4.append(mai)

        def emit_ma(b):
            mat = ma4[b % 4]
            t0 = 4 * b
            nc.sync.dma_start(out=mat[0:1, :], in_=x1T_t[t0:t0 + 4, 0:512])
            nc.sync.dma_start(out=mat[64:65, :], in_=x1T_t[t0:t0 + 4, 512:1024])
            nc.sync.dma_start(out=mat[1:2, :], in_=mT_t[t0:t0 + 4, 0:512])
            nc.sync.dma_start(out=mat[65:66, :], in_=mT_t[t0:t0 + 4, 512:1024])
            nc.gpsimd.dma_start(out=mat[2:3, :], in_=gxT[t0:t0 + 4, 0:512])
            nc.gpsimd.dma_start(out=mat[66:67, :], in_=gxT[t0:t0 + 4, 512:1024])
            nc.gpsimd.dma_start(out=mat[3:4, :],
                                in_=gxT[64 + t0:64 + t0 + 4, 0:512])
            nc.gpsimd.dma_start(out=mat[67:68, :],
                                in_=gxT[64 + t0:64 + t0 + 4, 512:1024])

        emit_ma(0)
        emit_ma(1)

        # ---- streaming pipeline -----------------------------------------
        Xs = {}
        tts = {}
        rts = {}
        gcur = {}

        def st_A(g):
            X = Xp.tile([128, 1024], f32, tag="X", name=f"X{g}")
            for h in range(2):
                p = 2 * g + h
                mat = ma4[(p // 4) % 4]
                mc = (p % 4) * CHUNK
                nc.tensor.matmul(
                    X[:, 512 * h:512 * h + 512], sa_t[:],
                    mat[:, mc:mc + CHUNK], start=True, stop=True)
            Xs[g] = X

        def st_t(g):
            tt = ttp.tile([128, 1024], h16, tag="tt", name=f"tt{g}")
            nc.scalar.activation(tt[:], Xs[g][:], AF.Prelu,
                                 bias=0.0, scale=1.0, alpha=a_slope)
            tts[g] = tt

        def st_B(g):
            X, tt = Xs[g], tts[g]
            for h in range(2):
                nc.tensor.matmul(
                    X[:, 512 * h:512 * h + 512], sb_t[:],
                    tt[:, 512 * h:512 * h + 512], start=True, stop=True)
            del tts[g]

        def st_rr(g):
            rt = rtp.tile([128, 1024], h16, tag="rt", name=f"rt{g}")
            nc.vector.tensor_scalar(
                out=rt[:], in0=Xs[g][:],
                scalar1=bk2_t[:, 0:1], scalar2=0.0,
                op0=ALU.add, op1=ALU.max)
            rts[g] = rt
            del Xs[g]

        def st_C(g):
            rt = rts[g]
            for h in range(2):
                p = 2 * g + h
                s = p // NSC
                j = p % NSC
                if j == 0:
                    gcur[s] = gp.tile([128, 512], f32, tag="go",
                                      name=f"go{s}")
                gam = gcur[s]
                nc.tensor.matmul(
                    gam[:], sc_t[:, 128 * j:128 * j + 128],
                    rt[:, 512 * h:512 * h + 512],
                    start=(j == 0), stop=(j == NSC - 1),
                    skip_group_check=True)
                if j == NSC - 1:
                    osb = osp.tile([128, 512], f32, tag="os", name=f"os{s}")
                    nc.scalar.copy(osb[0:64, :], gam[0:64, :])
                    nc.sync.dma_start(out=out_d[64 * s:64 * s + 64, :],
                                      in_=osb[0:64, :])
                    del gcur[s]
            del rts[g]

        # lags: B at g-2, C at g-4 -> every PE dep has ~2 iterations of
        # slack, so the PE FIFO never head-of-line blocks on ACT/DVE and
        # the HAM clock gate stays warm.
        for g in range(NGRP + 4):
            if g < NGRP:
                if g % 2 == 0 and g // 2 + 2 < NBLK:
                    emit_ma(g // 2 + 2)
                st_A(g)
                st_t(g)
            if 2 <= g < NGRP + 2:
                st_B(g - 2)
                st_rr(g - 2)
            if g >= 4:
                st_C(g - 4)

    nc.compile()
    return nc


def _get_program(a_slope, bfs_val):
    if "prog" not in _CACHE:
        _CACHE["prog"] = _build_program(a_slope, bfs_val)
    return _CACHE["prog"]


def _make_in_maps(x, mask_f, adj16, folded):
    shared = dict(adj=adj16, cge=folded["cge"], cgo=folded["cgo"],
                  ones=folded["ones"],
                  sa=folded["sa"], sb=folded["sb"], sc=folded["sc"],
                  bk2=folded["bk2"])
    in_maps = []
    for b in range(B):
        xb = np.ascontiguousarray(x[b, 0]).astype(np.float16)   # [N, T]
        mb = np.ascontiguousarray(mask_f[b, 0])                 # [N, T] f16
        m = dict(shared)
        m["xT"] = np.ascontiguousarray(xb.T)
        m["mT"] = np.ascontiguousarray(mb.T)
        m["xn"] = np.ascontiguousarray(
            xb.reshape(8, 128, T).transpose(1, 0, 2).reshape(128, 512))
        m["mn"] = np.ascontiguousarray(
            mb.reshape(8, 128, T).transpose(1, 0, 2).reshape(128, 512))
        in_maps.append(m)
    return in_maps


def kernel(x, mask, W_fs, b_fs, W_in, b_in, adj, W_gc, b_gc, W_lo, b_lo,
           prelu_a, W_ro, b_ro, W_o1, b_o1, W_o2, b_o2):
    x = np.asarray(x, np.float32)
    mask_f = np.asarray(mask, np.float16)
    adj = np.asarray(adj, np.float32)

    folded = _fold_weights(np.asarray(W_fs), np.asarray(b_fs),
                           np.asarray(W_in), np.asarray(b_in),
                           np.asarray(W_gc), np.asarray(b_gc),
                           np.asarray(W_lo), np.asarray(b_lo),
                           float(prelu_a),
                           np.asarray(W_ro), np.asarray(b_ro),
                           np.asarray(W_o1), np.asarray(b_o1),
                           np.asarray(W_o2), np.asarray(b_o2), adj)

    nc = _get_program(folded["a"], folded["bfs"])
    in_maps = _make_in_maps(x, mask_f, adj.astype(np.float16), folded)

    from concourse.bass_utils import run_bass_kernel_spmd
    res = run_bass_kernel_spmd(nc, in_maps, list(range(B)))

    out = np.empty((B, C, N, T), np.float32)
    for b in range(B):
        # out row r = chunk r; chunk 2p+h = (t=p, n-half h)
        ob = np.asarray(res.results[b]["out"]).reshape(T, 2, 512)
        ob = ob.transpose(0, 1, 2).reshape(T, N)   # [t, n]
        out[b, 0] = ob.T + folded["b_o2"]
    return out
